# revision 52
# baseline (speedup 1.0000x reference)
"""DigitCaps dynamic-routing kernel for Trainium2 (Bass/Tile), 8 NeuronCores.

Problem:  u_hat[b,d,n,v] = sum_q W[d,n,v,q] * u[b,n,q]
          3 routing iterations of  c = softmax_d(b);  s = sum_n c*u_hat;
          v = squash(s);  b += u_hat . v
Shapes:   B=32, N=32768, Q=8, D=10, V=16.

Strategy: shard N across the 8 cores (N_loc=4096).  u_hat (671 MB) is never
materialized; each routing pass is recomputed from SBUF-resident W via PE
matmuls.  Because  b_r = u_hat . (v_0 + ... + v_{r-1}),  the logits are
rebuilt each pass from the running vsum, so no cross-pass state lives on
device.  Each launch returns the per-core partial contraction; the host does
the tiny [B,D,V] reduction + squash between the three launches.

Per-core pass structure, processed in 4 octets of 8 chunks (128 n's each):
  stage B1 (passes 1,2): wv[(qin,b); n] = sum_v vsum[b,d,v] W[d,n,v,q] via
     one PE matmul per (d, q-half), K=64 block-diag-of-4 vsum operand.
  consume: tmp = wv * u  (DVE tensor_tensor, PSUM in).
  MM2: logits[n128, (d,b)] += tmp.T @ sel  (PE; sel[(qin,b),b']=delta_bb'
     transposes to n-partitions and sums over q; accumulates the 2 q-halves).
  softmax over d: ACT exp from PSUM, DVE add chain over d, reciprocal, mult.
  stage A: cu[n,(q,b)] = c_d[n,b] * u[n,(q,b)] (DVE, q-broadcast);
     pa[(q',v)=128, (q,b)=256] += W_A[d,t][n,(q',v)].T @ cu  (PE, accumulated
     over the octet's 8 chunks, then dumped PSUM->DRAM per (octet, d)).
  Host: s[d,v,b] = sum_oct sum_q pa[oct, d, q*16+v, q*32+b], reduce cores.

Pass 0 has uniform c = 1/D: stage A runs directly on u (1/D folded on host).
"""

import os
import numpy as np
import ml_dtypes

import concourse.bass as bass
import concourse.tile as tile
from concourse import mybir
from concourse.bass_utils import run_bass_kernel_spmd

BF16 = mybir.dt.bfloat16
F32 = mybir.dt.float32
F16 = mybir.dt.float16
NPBF16 = ml_dtypes.bfloat16

B, N, Q = 32, 32768, 8
D, V = 10, 16
NCORES = 8
NLOC = N // NCORES            # 4096
NT = NLOC // 128              # 32 chunks of 128 n's
NOCT = 8                      # groups of chunks
TPO = NT // NOCT              # 8 chunks per octet
EPS = 1e-7
NUM_ROUTINGS = 3


# ----------------------------------------------------------------------------
# Bass programs
# ----------------------------------------------------------------------------

def build_prog(uniform: bool) -> bass.Bass:
    nc = bass.Bass(target_bir_lowering=False)

    # [n128-part, d, t, (q'*16+v)=128]
    w_a = nc.declare_dram_parameter("w_a", [128, D, NT, 128], BF16, isOutput=False)
    # [n128-part, t, (q*32+b)=256]
    u_qb = nc.declare_dram_parameter("u_qb", [128, NT, 256], BF16, isOutput=False)
    # raw psum accumulation: [(q'8,v16)=128, d, 256]
    s_out = nc.declare_dram_parameter("s_out", [128, D, 256], F16,
                                      isOutput=True)
    if not uniform:
        # [(g2,qin4,v16)=128-part, d, n4096]
        w_b = nc.declare_dram_parameter("w_b", [128, D, NLOC], BF16, isOutput=False)
        # [(qin4,b32)=128-part, g2, n4096]
        u_qbn = nc.declare_dram_parameter("u_qbn", [128, 2, NLOC], BF16,
                                          isOutput=False)
        # block-diag-of-4 vsum, replicated on both 64-partition halves:
        # [d, (half2,j4,v16)=128, (qp4,b32)=128]
        vs_bd = nc.declare_dram_parameter("vs_bd", [D, 128, 128], BF16,
                                          isOutput=False)
        # [(qin4,b32)=128, b'32]
        sel = nc.declare_dram_parameter("sel", [128, B], BF16, isOutput=False)

    with tile.TileContext(nc, linearize=True) as tc:
        with (
            tc.tile_pool(name="weights", bufs=1) as weights,
            tc.tile_pool(name="consts", bufs=1) as consts,
            tc.tile_pool(name="work", bufs=2) as work,
            tc.tile_pool(name="softmax", bufs=1) as softmax_pool,
            tc.tile_pool(name="psum_a", bufs=2, space=bass.MemorySpace.PSUM) as psum_a_pool,
            tc.tile_pool(name="psum_b", bufs=2, space=bass.MemorySpace.PSUM) as psum_b_pool,
            tc.tile_pool(name="psum_l", bufs=1, space=bass.MemorySpace.PSUM) as psum_l_pool,
        ):
            # ---- resident loads ----
            sb_wa = weights.tile([128, D, NT, 128], BF16)
            for d in range(D):
                nc.sync.dma_start(out=sb_wa[:, d], in_=w_a[:, d])
            sb_uqb = weights.tile([128, NT, 256], BF16)
            nc.sync.dma_start(out=sb_uqb, in_=u_qb[:])
            if not uniform:
                sb_wb = weights.tile([128, D, NLOC], BF16)
                for d in range(D):
                    nc.sync.dma_start(out=sb_wb[:, d], in_=w_b[:, d])
                sb_uqbn = weights.tile([128, 2, NLOC], BF16)
                nc.sync.dma_start(out=sb_uqbn, in_=u_qbn[:])
                sb_vsbd = consts.tile([128, D, 128], BF16)
                nc.sync.dma_start(out=sb_vsbd,
                                  in_=vs_bd[:].rearrange("d k m -> k d m"))
                sb_sel = consts.tile([128, B], BF16)
                nc.sync.dma_start(out=sb_sel, in_=sel[:])
                ct_oct = weights.tile([128, TPO, D, B], BF16)
            sb_acc = weights.tile([128, D, 256], F16)

            for oct_i in range(NOCT):
                # ---------- phase 1: routing coefficients for this octet ----
                for tt in range(TPO if not uniform else 0):
                    t = oct_i * TPO + tt
                    psum_l = psum_l_pool.tile([128, D, B], F32)   # [n, d, b]
                    for d in range(D):
                        for g in range(2):
                            psum_wv = psum_b_pool.tile([128, 128], F32)
                            po = g * 64
                            nc.tensor.matmul(
                                psum_wv[:, :],
                                sb_vsbd[po:po + 64, d, :],          # [64, 128]
                                sb_wb[po:po + 64,
                                      d, t * 128:(t + 1) * 128],    # [64, 128]
                                start=True, stop=True,
                            )
                            tmp = work.tile([128, 128], BF16, tag="tmp")
                            nc.vector.tensor_mul(
                                tmp[:, :], psum_wv[:, :],
                                sb_uqbn[:, g, t * 128:(t + 1) * 128],
                            )
                            nc.tensor.matmul(
                                psum_l[:, d, :],
                                tmp[:, :],                          # [128, 128]
                                sb_sel[:, :],                       # [128, 32]
                                start=(g == 0), stop=(g == 1),
                            )
                    # ---------- softmax over d ----------
                    et = softmax_pool.tile([128, D, B], F32, tag="exp")
                    # 1-elem ACT dummy write absorbs the slot-reuse WAR wait
                    # so the Exp carries at most 2 sync waits (walrus limit).
                    nc.scalar.activation(et[:, 0, 0:1], et[:, 0, 0:1],
                                         mybir.ActivationFunctionType.Copy)
                    nc.scalar.activation(et[:], psum_l[:],
                                         mybir.ActivationFunctionType.Exp)
                    zt = softmax_pool.tile([128, B], F32, tag="z")
                    nc.vector.tensor_add(zt[:], et[:, 0, :], et[:, 1, :])
                    for d in range(2, D):
                        nc.vector.tensor_add(zt[:], zt[:], et[:, d, :])
                    rz = softmax_pool.tile([128, B], F32, tag="rz")
                    nc.vector.reciprocal(rz[:], zt[:])
                    nc.vector.tensor_mul(
                        ct_oct[:, tt], et[:],
                        rz[:, None, :].broadcast_to([128, D, B]),
                    )

                # ---------- phase 2: stage A for this octet, d-outer --------
                for d in range(D):
                    pa = psum_a_pool.tile([128, 256], F32)
                    for tt in range(TPO):
                        t = oct_i * TPO + tt
                        if uniform:
                            rhs = sb_uqb[:, t, :]
                        else:
                            cu = work.tile([128, Q, B], BF16, tag="cu")
                            nc.vector.tensor_mul(
                                cu[:],
                                sb_uqb[:, t, :].rearrange("n (q b) -> n q b",
                                                          q=Q),
                                ct_oct[:, tt, d, :][:, None, :]
                                .broadcast_to([128, Q, B]),
                            )
                            rhs = cu[:].rearrange("n q b -> n (q b)")
                        nc.tensor.matmul(
                            pa[:, :],
                            sb_wa[:, d, t, :],                      # [n128, 128]
                            rhs,                                    # [n128, 256]
                            start=(tt == 0), stop=(tt == TPO - 1),
                        )
                    # accumulate into the SBUF staging accumulator
                    if oct_i == 0:
                        nc.vector.tensor_copy(sb_acc[:, d, :], pa[:, :])
                    else:
                        nc.vector.tensor_add(sb_acc[:, d, :], sb_acc[:, d, :],
                                             pa[:, :])
            nc.gpsimd.dma_start(out=s_out[:], in_=sb_acc[:])

    return nc


# ----------------------------------------------------------------------------
# Host-side packing
# ----------------------------------------------------------------------------

def pack_static(W, u):
    """Per-core static input packing. W:[D,N,V,Q] f32, u:[B,N,Q] f32."""
    per_core = []
    for c in range(NCORES):
        sl = slice(c * NLOC, (c + 1) * NLOC)
        Wc = W[:, sl]                      # [D, NLOC, V, Q]
        uc = u[:, sl]                      # [B, NLOC, Q]
        # w_a[i, d, t, q'*16+v] = Wc[d, t*128+i, v, q']
        w_a = np.ascontiguousarray(
            Wc.reshape(D, NT, 128, V, Q).transpose(2, 0, 1, 4, 3)
            .reshape(128, D, NT, 128).astype(NPBF16)
        )
        # w_b[g*64+qin*16+v, d, n] = Wc[d, n, v, 4g+qin]
        w_b = np.ascontiguousarray(
            Wc.transpose(3, 2, 0, 1)       # [Q, V, D, NLOC]
            .reshape(2, 4, V, D, NLOC).reshape(128, D, NLOC).astype(NPBF16)
        )
        # u_qb[i, t, q*32+b] = uc[b, t*128+i, q]
        u_qb = np.ascontiguousarray(
            uc.reshape(B, NT, 128, Q).transpose(2, 1, 3, 0)
            .reshape(128, NT, 256).astype(NPBF16)
        )
        # u_qbn[qin*32+b, g, n] = uc[b, n, 4g+qin]
        u_qbn = np.ascontiguousarray(
            uc.transpose(2, 0, 1)          # [Q, B, NLOC]
            .reshape(2, 4, B, NLOC).transpose(1, 2, 0, 3)
            .reshape(128, 2, NLOC).astype(NPBF16)
        )
        per_core.append({"w_a": w_a, "w_b": w_b, "u_qb": u_qb, "u_qbn": u_qbn})
    return per_core


def pack_vsum(vsum):
    """vsum [B,D,V] f32 -> block-diag-of-4, 2x partition-replicated
    [D, 128, 128] bf16."""
    vsumT = vsum.transpose(2, 1, 0)        # [V, D, B]
    out = np.zeros((D, 64, 128), np.float32)
    for j in range(4):
        out[:, j * V:(j + 1) * V, j * B:(j + 1) * B] = vsumT.transpose(1, 0, 2)
    return np.tile(out, (1, 2, 1)).astype(NPBF16)


SEL = np.kron(np.ones((4, 1), np.float32), np.eye(B, dtype=np.float32)).astype(NPBF16)


def extract_s(parts):
    """parts: list of per-core s_out [128, D, 256] f32 -> s [B,D,V]."""
    tot = np.zeros((128, D, 256), np.float64)
    for p in parts:
        tot += p.reshape(128, D, 256).astype(np.float64)
    P4 = tot.reshape(Q, V, D, Q, B)        # [q', v, d, q, b]
    s = np.zeros((D, V, B), np.float64)
    for q in range(Q):
        s += P4[q, :, :, q, :].transpose(1, 0, 2)
    return np.ascontiguousarray(s.transpose(2, 0, 1)).astype(np.float32)


def squash_np(s):
    norm = np.linalg.norm(s, axis=-1, keepdims=True)
    coef = norm ** 2 / (norm ** 2 + 1.0)
    return coef * s / (norm + EPS)


# ----------------------------------------------------------------------------
# Entry point
# ----------------------------------------------------------------------------

_PROGS: dict = {}
LAST_EXEC_NS = None
LAST_EXEC_DETAIL = None


def _get_prog(uniform: bool) -> bass.Bass:
    if uniform not in _PROGS:
        _PROGS[uniform] = build_prog(uniform)
    return _PROGS[uniform]


# ----------------------------------------------------------------------------
# Primary path: the same N-sharded routing algorithm, compiled by neuron-XLA
# via shard_map over the 8 NeuronCores.  (The hand-written Bass pipeline above
# is kept for reference / future use behind DIGITCAP_BASS=1: the walrus build
# on this toolchain rejects any instruction carrying more than one sync wait,
# which Tile-emitted schedules cannot currently guarantee.)
# ----------------------------------------------------------------------------

# O1 keeps neuronx-cc compile time bounded (O2 default ran >9 min on this
# HLO); set before the first jax/backend touch so the flag reaches the
# compiler.
os.environ.setdefault("NEURON_CC_FLAGS", "--optlevel=1")

import jax
import jax.numpy as jnp
from jax.sharding import Mesh, PartitionSpec as _P
from jax.experimental.shard_map import shard_map as _shard_map


def _squash_jnp(s):
    norm = jnp.linalg.norm(s, axis=-1, keepdims=True)
    coef = norm ** 2 / (norm ** 2 + 1.0)
    return coef * s / (norm + EPS)


def _routing_local(u_loc, W_loc):
    # u_loc: [B, NLOC, Q], W_loc: [D, NLOC, V, Q] — one core's N-shard.
    u_hat = jnp.einsum("dnvq,bnq->bdnv", W_loc, u_loc)
    b = jnp.zeros(u_hat.shape[:3], u_hat.dtype)
    v = None
    for r in range(NUM_ROUTINGS):
        c = jax.nn.softmax(b, axis=1)
        s = jnp.einsum("bdn,bdnv->bdv", c, u_hat)
        s = jax.lax.psum(s, axis_name="core")     # tiny [B,D,V] all-reduce
        v = _squash_jnp(s)
        if r < NUM_ROUTINGS - 1:
            b = b + jnp.einsum("bdnv,bdv->bdn", u_hat, v)
    return v


_JAX_COMPILED = []


def _jax_kernel(primary_caps: np.ndarray, W: np.ndarray) -> np.ndarray:
    if not _JAX_COMPILED:
        mesh = Mesh(np.asarray(jax.devices()[:NCORES]), ("core",))
        fn = _shard_map(
            _routing_local, mesh=mesh,
            in_specs=(_P(None, "core", None), _P(None, "core", None, None)),
            out_specs=_P(None, None, None),
            check_rep=False,
        )
        _JAX_COMPILED.append(jax.jit(fn))
    out = _JAX_COMPILED[0](jnp.asarray(primary_caps, jnp.float32),
                           jnp.asarray(W, jnp.float32))
    return np.asarray(jax.device_get(out)).astype(np.float32)


def kernel(primary_caps: np.ndarray, W: np.ndarray) -> np.ndarray:
    if not os.environ.get("DIGITCAP_BASS"):
        return _jax_kernel(primary_caps, W)
    return _bass_kernel(primary_caps, W)


def _bass_kernel(primary_caps: np.ndarray, W: np.ndarray) -> np.ndarray:
    global LAST_EXEC_NS, LAST_EXEC_DETAIL
    trace = bool(os.environ.get("DIGITCAP_TRACE"))
    u = np.asarray(primary_caps, np.float32)
    Wf = np.asarray(W, np.float32)
    static = pack_static(Wf, u)
    core_ids = list(range(NCORES))
    exec_ns = []

    vsum = np.zeros((B, D, V), np.float32)
    v = None
    for r in range(NUM_ROUTINGS):
        if r == 0:
            prog = _get_prog(True)
            in_maps = [
                {"w_a": s["w_a"], "u_qb": s["u_qb"]} for s in static
            ]
        else:
            prog = _get_prog(False)
            vs_bd = pack_vsum(vsum)
            in_maps = [
                {"w_a": s["w_a"], "w_b": s["w_b"], "u_qb": s["u_qb"],
                 "u_qbn": s["u_qbn"], "vs_bd": vs_bd, "sel": SEL}
                for s in static
            ]
        res = run_bass_kernel_spmd(prog, in_maps, core_ids, trace=trace)
        if res.exec_time_ns:
            exec_ns.append(res.exec_time_ns)
        parts = [res.results[c]["s_out"] for c in range(NCORES)]
        s = extract_s(parts)               # [B, D, V]
        if r == 0:
            s = s / D
        v = squash_np(s)
        vsum = vsum + v
    if exec_ns:
        LAST_EXEC_NS = int(sum(exec_ns))
        LAST_EXEC_DETAIL = exec_ns
    return v.astype(np.float32)
